# revision 9
# baseline (speedup 1.0000x reference)
"""CTC compressor (weighted strategy) for Trainium2 — Bass/Tile kernel.

Problem: B=8, T=2048, D=1024, V=1024.
  probs = softmax(ctc_logits); pred = argmax(ctc_logits)
  segments = runs of equal non-blank pred within length; per-frame weight
  p[t] = probs[t, pred[t]] normalized within segment; output[s] = weighted
  sum of hidden over frames of segment s (zero-padded to T rows).

Key identity used here: out[s] = (sum_{t in seg s} p~[t] * h[t]) / (sum p~ + eps)
with p~ = p * frame_in_seg.  Segments are contiguous frame runs, so the
segment sums are differences of a global cumulative sum along T:
  S[s] = CE[a[s+1]] - CE[a[s]],  CE[t] = sum_{tau<t} p~ h,  a[s] = start of seg s.
This replaces the reference's dense (T x T') x (T x D) matmul (8.6 GFLOP/core)
with: softmax stats + a hardware prefix-scan + an indirect row gather + a
banded-diff matmul, all memory-bound.

Sharding: pure data parallel — one batch element per NeuronCore (8 cores).
"""

import numpy as np
from contextlib import ExitStack

import concourse.bass as bass
import concourse.bacc as bacc
import concourse.mybir as mybir
import concourse.tile as tile
from concourse.bass import IndirectOffsetOnAxis
from concourse.bass_utils import run_bass_kernel_spmd
from concourse.masks import make_identity

F32 = mybir.dt.float32
I32 = mybir.dt.int32
U32 = mybir.dt.uint32
AF = mybir.ActivationFunctionType
OP = mybir.AluOpType

T, D, V = 2048, 1024, 1024
P = 128
NT = T // P            # 16 t-chunks
ND = D // P            # 8 d-chunks
PCOL = D               # column index of the p~ cumsum in the CE table
TW = D + 16            # CE table row width (1024 dims + p-col + 15 pad), 64B-aligned
CEROWS = T + 1         # row 0 = zeros, row 1+t = inclusive cumsum through frame t
AROWS = 4224           # segment-start table; >= TRASH + T
TRASH = 2064.0         # masked scatter targets go to rows TRASH + t (unique, unread)
EPS = 1e-10
NCORES = 8


def _build_body(ctx, tc, nc, lg, hs, lenb, out, nlen):
    sbc = ctx.enter_context(tc.tile_pool(name="sbc", bufs=1))
    sb = ctx.enter_context(tc.tile_pool(name="sb", bufs=2))
    sbio = ctx.enter_context(tc.tile_pool(name="sbio", bufs=3))
    sbz = ctx.enter_context(tc.tile_pool(name="sbz", bufs=1))
    sbg = ctx.enter_context(tc.tile_pool(name="sbg", bufs=4))
    pt = ctx.enter_context(tc.tile_pool(name="pt", bufs=2, space="PSUM"))
    ps = ctx.enter_context(tc.tile_pool(name="ps", bufs=1, space="PSUM"))
    pm = ctx.enter_context(tc.tile_pool(name="pm", bufs=1, space="PSUM"))
    dram = ctx.enter_context(tc.tile_pool(name="dram", bufs=1, space="DRAM"))

    # ---- constants (built on device, no host inputs) ----
    ident = sbc.tile([P, P], F32, tag="ident")
    make_identity(nc, ident[:])

    # adiff[p, r] = +1 if p == r+1, -1 if p == r  -> out[r] = G[r+1] - G[r]
    adiff = sbc.tile([P, P], F32, tag="adiff")
    nc.gpsimd.memset(adiff[:], 0.0)
    nc.gpsimd.affine_select(out=adiff[:], in_=adiff[:], compare_op=OP.not_equal,
                            fill=-1.0, base=0, pattern=[[-1, P]], channel_multiplier=1)
    nc.gpsimd.affine_select(out=adiff[:], in_=adiff[:], compare_op=OP.not_equal,
                            fill=1.0, base=-1, pattern=[[-1, P]], channel_multiplier=1)

    # frame index t = p + 128*c as int32 and f32
    tiota = sbc.tile([P, NT], I32, tag="tiota")
    nc.gpsimd.iota(tiota[:], pattern=[[P, NT]], base=0, channel_multiplier=1)
    tiotaf = sbc.tile([P, NT], F32, tag="tiotaf")
    nc.vector.tensor_copy(tiotaf[:], tiota[:])

    lent = sbc.tile([P, 1], F32, tag="lent")
    nc.sync.dma_start(lent[:], lenb[:])

    # ---- DRAM scratch ----
    cet = dram.tile([CEROWS, TW], F32)      # cumulative-sum gather table
    at = dram.tile([AROWS, 1], I32)         # segment start frames

    # init a-table to T (sentinel: "start = end of sequence")
    ainit = sbc.tile([P, AROWS // P], I32, tag="ainit")
    nc.gpsimd.memset(ainit[:], T)
    nc.sync.dma_start(at[:, :], ainit[:])

    # zero row 0 of CE table (exclusive cumsum at t=0)
    zrow = sbc.tile([1, TW], F32, tag="zrow")
    nc.gpsimd.memset(zrow[:], 0.0)
    nc.sync.dma_start(cet[0:1, :], zrow[:])

    # ---- phase 1: softmax stats + argmax per frame ----
    mcol = sbc.tile([P, NT], F32, tag="mcol")    # negated per-frame max logit
    dns = sbc.tile([P, NT], F32, tag="dns")      # sum exp(logit - max)
    pcol = sbc.tile([P, NT], F32, tag="pcol")    # p = 1/dns
    predf = sbc.tile([P, NT], F32, tag="predf")  # argmax as f32
    l00 = sbc.tile([1, 1], F32, tag="l00")

    for i in range(NT):
        lgt = sbio.tile([P, V], F32, tag="lgt")
        nc.sync.dma_start(lgt[:], lg[P * i:P * (i + 1), :])
        mx8 = sb.tile([P, 8], F32, tag="mx8")
        nc.vector.max(mx8[:], lgt[:])
        ix8 = sb.tile([P, 8], U32, tag="ix8")
        nc.vector.max_index(ix8[:], mx8[:], lgt[:])
        nc.vector.tensor_scalar_mul(mcol[:, i:i + 1], mx8[:, 0:1], -1.0)
        nc.vector.tensor_copy(predf[:, i:i + 1], ix8[:, 0:1])
        nc.scalar.activation(lgt[:], lgt[:], AF.Exp, bias=mcol[:, i:i + 1],
                             scale=1.0, accum_out=dns[:, i:i + 1])
        nc.vector.reciprocal(pcol[:, i:i + 1], dns[:, i:i + 1])
        if i == 0:
            nc.vector.tensor_copy(l00[:], lgt[0:1, 0:1])

    # ---- phase 2: masks, p~, run boundaries ----
    prevf = sbc.tile([P, NT], F32, tag="prevf")
    nc.sync.dma_start(prevf[1:P, :], predf[0:P - 1, :])
    nc.sync.dma_start(prevf[0:1, 1:NT], predf[P - 1:P, 0:NT - 1])
    nc.gpsimd.memset(prevf[0:1, 0:1], -1.0)

    valid = sbc.tile([P, NT], F32, tag="valid")
    nc.vector.tensor_scalar(valid[:], tiotaf[:], lent[:, 0:1], None, op0=OP.is_lt)
    neq = sb.tile([P, NT], F32, tag="neq")
    nc.vector.tensor_tensor(neq[:], predf[:], prevf[:], op=OP.not_equal)
    nblank = sb.tile([P, NT], F32, tag="nblank")
    nc.vector.tensor_scalar(nblank[:], predf[:], 0.0, None, op0=OP.not_equal)
    bnd = sb.tile([P, NT], F32, tag="bnd")
    nc.vector.tensor_tensor(bnd[:], neq[:], valid[:], op=OP.mult)
    nb = sbc.tile([P, NT], F32, tag="nb")
    nc.vector.tensor_tensor(nb[:], bnd[:], nblank[:], op=OP.mult)
    fis = sb.tile([P, NT], F32, tag="fis")
    nc.vector.tensor_tensor(fis[:], valid[:], nblank[:], op=OP.mult)
    ptil = sbc.tile([P, NT], F32, tag="ptil")
    nc.vector.tensor_tensor(ptil[:], pcol[:], fis[:], op=OP.mult)

    # ---- transpose p~ and nb to a single free-dim row, prefix-scan them ----
    pnb = sb.tile([P, 2 * NT], F32, tag="pnb")
    nc.vector.tensor_copy(pnb[:, 0:NT], ptil[:])
    nc.vector.tensor_copy(pnb[:, NT:2 * NT], nb[:])
    ps32 = pm.tile([2 * NT, P], F32, tag="ps32")
    nc.tensor.transpose(ps32[:], pnb[:], ident[:])
    pnbT = sb.tile([2 * NT, P], F32, tag="pnbT")
    nc.vector.tensor_copy(pnbT[:], ps32[:])

    prow = sbc.tile([1, T], F32, tag="prow")
    nrow = sbc.tile([1, T], F32, tag="nrow")
    nc.sync.dma_start(prow[:], pnbT[0:NT, :])        # fold [16,128] -> [1,2048]
    nc.sync.dma_start(nrow[:], pnbT[NT:2 * NT, :])

    pcrow = sbc.tile([1, T], F32, tag="pcrow")       # inclusive cumsum of p~
    nc.vector.tensor_tensor_scan(pcrow[:], prow[:], prow[:], initial=0.0,
                                 op0=OP.add, op1=OP.bypass)
    scrow = sbc.tile([1, T], F32, tag="scrow")       # inclusive cumsum of nb
    nc.vector.tensor_tensor_scan(scrow[:], nrow[:], nrow[:], initial=0.0,
                                 op0=OP.add, op1=OP.bypass)

    # unfold scrow back to [128, 16] chunk-column layout (p inner, c outer)
    seg16 = sbc.tile([P, NT], F32, tag="seg16")
    for c in range(NT):
        nc.sync.dma_start(seg16[:, c:c + 1], scrow[0:1, P * c:P * (c + 1)])

    # ---- scatter segment start frames: a[seg_idx[t]] = t where nb[t] ----
    si = sb.tile([P, NT], F32, tag="si")
    nc.vector.tensor_scalar_add(si[:], seg16[:], -1.0)     # seg_idx = cumsum-1
    tr = sb.tile([P, NT], F32, tag="tr")
    nc.vector.tensor_scalar_add(tr[:], tiotaf[:], TRASH)   # unique trash rows
    # idx = tr + nb*(si - tr)   (nb is 0/1; avoids CopyPredicated int-mask rule)
    idxf = sb.tile([P, NT], F32, tag="idxf")
    nc.vector.tensor_tensor(idxf[:], si[:], tr[:], op=OP.subtract)
    nc.vector.tensor_tensor(idxf[:], idxf[:], nb[:], op=OP.mult)
    nc.vector.tensor_tensor(idxf[:], idxf[:], tr[:], op=OP.add)
    idxi = sbc.tile([P, NT], I32, tag="idxi")
    nc.vector.tensor_copy(idxi[:], idxf[:])
    for c in range(NT):
        nc.gpsimd.indirect_dma_start(
            out=at[:, :],
            out_offset=IndirectOffsetOnAxis(ap=idxi[:, c:c + 1], axis=0),
            in_=tiota[:, c:c + 1],
            in_offset=None)

    # read back a[0:2176] as [128, 17] (p inner, c outer)
    asb = sbc.tile([P, NT + 1], I32, tag="asb")
    nc.sync.dma_start(
        asb[:], at[0:P * (NT + 1), 0:1].rearrange("(c p) one -> p c one", p=P))

    # ---- phase 3: z = p~ * h ; transpose; global prefix scan along T ----
    zT = sbz.tile([P, ND * T], F32, tag="zT")   # [d-part, t-free], 8 d-chunks
    for i in range(NT):
        ht = sbio.tile([P, D], F32, tag="ht")
        nc.sync.dma_start(ht[:], hs[P * i:P * (i + 1), :])
        if i == 0:
            hrow0 = sbc.tile([1, D], F32, tag="hrow0")
            nc.vector.tensor_copy(hrow0[:], ht[0:1, :])
        nc.vector.tensor_scalar_mul(ht[:], ht[:], ptil[:, i:i + 1])
        for g in range(2):
            tp = pt.tile([P, 512], F32, tag="tp")
            for q in range(4):
                j = 4 * g + q
                nc.tensor.transpose(tp[:, P * q:P * (q + 1)],
                                    ht[:, P * j:P * (j + 1)], ident[:])
            dst = zT[:].rearrange("p (j t) -> p j t", j=ND)[:, 4 * g:4 * g + 4,
                                                           P * i:P * (i + 1)]
            nc.vector.tensor_copy(dst, tp[:].rearrange("p (q t) -> p q t", q=4))

    for j in range(ND):
        nc.vector.tensor_tensor_scan(
            zT[:, T * j:T * (j + 1)], zT[:, T * j:T * (j + 1)],
            zT[:, T * j:T * (j + 1)], initial=0.0, op0=OP.add, op1=OP.bypass)

    # ---- transpose back into CE rows, append p~ cumsum col, store to DRAM ----
    for i in range(NT):
        ce = sbio.tile([P, TW], F32, tag="ce")
        for g in range(2):
            tp2 = pt.tile([P, 512], F32, tag="tp2")
            for q in range(4):
                j = 4 * g + q
                nc.tensor.transpose(tp2[:, P * q:P * (q + 1)],
                                    zT[:, T * j + P * i:T * j + P * (i + 1)],
                                    ident[:])
            nc.scalar.copy(ce[:, 512 * g:512 * (g + 1)], tp2[:])
        nc.gpsimd.memset(ce[:, PCOL:TW], 0.0)
        nc.sync.dma_start(ce[:, PCOL:PCOL + 1], pcrow[0:1, P * i:P * (i + 1)])
        nc.sync.dma_start(cet[1 + P * i:1 + P * (i + 1), :], ce[:])

    # ---- phase 4/5: gather G[s] = CE[a[s]], band-diff, normalize, store ----
    gts = []
    for k in range(NT + 1):
        gt = sbg.tile([P, TW], F32, tag="gt")
        nc.gpsimd.indirect_dma_start(
            out=gt[:], out_offset=None, in_=cet[:, :],
            in_offset=IndirectOffsetOnAxis(ap=asb[:, k:k + 1], axis=0))
        gts.append(gt)

    # fallback scalars: flag = (n_seg == 0) & (len >= 1); coef = p0/(p0+eps)
    nsegv = scrow[0:1, T - 1:T]
    e00 = sb.tile([1, 1], F32, tag="e00")
    nc.scalar.activation(e00[:], l00[:], AF.Exp, bias=mcol[0:1, 0:1], scale=1.0)
    p0 = sb.tile([1, 1], F32, tag="p0")
    nc.vector.tensor_tensor(p0[:], e00[:], pcol[0:1, 0:1], op=OP.mult)
    pe0 = sb.tile([1, 1], F32, tag="pe0")
    nc.vector.tensor_scalar_add(pe0[:], p0[:], EPS)
    per0 = sb.tile([1, 1], F32, tag="per0")
    nc.vector.reciprocal(per0[:], pe0[:])
    fc = sb.tile([1, 1], F32, tag="fc")
    nc.vector.tensor_tensor(fc[:], p0[:], per0[:], op=OP.mult)
    e1 = sb.tile([1, 1], F32, tag="e1")
    nc.vector.tensor_scalar(e1[:], nsegv, 0.0, None, op0=OP.is_equal)
    e2 = sb.tile([1, 1], F32, tag="e2")
    nc.vector.tensor_scalar(e2[:], lent[0:1, 0:1], 1.0, None, op0=OP.is_ge)
    flag = sb.tile([1, 1], F32, tag="flag")
    nc.vector.tensor_tensor(flag[:], e1[:], e2[:], op=OP.mult)
    fcoef = sbc.tile([1, 1], F32, tag="fcoef")
    nc.vector.tensor_tensor(fcoef[:], fc[:], flag[:], op=OP.mult)
    fbrow = sbc.tile([1, D], F32, tag="fbrow")
    nc.vector.tensor_scalar_mul(fbrow[:], hrow0[:], fcoef[0:1, 0:1])

    # new_lengths = max(n_seg, 1)
    nlf = sb.tile([1, 1], F32, tag="nlf")
    nc.vector.tensor_scalar_max(nlf[:], nsegv, 1.0)
    nli = sb.tile([1, 1], I32, tag="nli")
    nc.vector.tensor_copy(nli[:], nlf[:])
    nc.sync.dma_start(nlen[:, :], nli[:])

    # compute-engine APs may only start at partitions 0/32/64/96: stage the
    # row-127 correction in a zeroed [96:128) strip and add that strip.
    gfixes = []
    for t in range(2):
        gf = sbc.tile([P, TW], F32, tag=f"gfix{t}")
        nc.gpsimd.memset(gf[96:P, :], 0.0)
        gfixes.append(gf)

    for k in range(NT):
        sp = ps.tile([P, TW], F32, tag="sp")
        nc.tensor.matmul(sp[:, 0:512], lhsT=adiff[:], rhs=gts[k][:, 0:512],
                         start=True, stop=True)
        nc.tensor.matmul(sp[:, 512:1024], lhsT=adiff[:], rhs=gts[k][:, 512:1024],
                         start=True, stop=True)
        nc.tensor.matmul(sp[:, 1024:TW], lhsT=adiff[:], rhs=gts[k][:, 1024:TW],
                         start=True, stop=True)
        gfix = gfixes[k % 2]
        nc.sync.dma_start(gfix[P - 1:P, :], gts[k + 1][0:1, :])
        nc.vector.tensor_tensor(sp[96:P, :], sp[96:P, :],
                                gfix[96:P, :], op=OP.add)
        radd = sb.tile([P, 1], F32, tag="radd")
        nc.vector.tensor_scalar_add(radd[:], sp[:, PCOL:PCOL + 1], EPS)
        rec = sb.tile([P, 1], F32, tag="rec")
        nc.vector.reciprocal(rec[:], radd[:])
        ot = sbio.tile([P, D], F32, tag="ot")
        nc.scalar.mul(ot[:], sp[:, 0:D], rec[:])
        if k == 0:
            nc.vector.tensor_tensor(ot[0:1, :], ot[0:1, :], fbrow[:], op=OP.add)
        nc.sync.dma_start(out[P * k:P * (k + 1), :], ot[:])


def build_nc():
    nc = bacc.Bacc("TRN2", target_bir_lowering=False, debug=False)
    lg = nc.dram_tensor("lg", [T, V], F32, kind="ExternalInput")
    hs = nc.dram_tensor("hs", [T, D], F32, kind="ExternalInput")
    lenb = nc.dram_tensor("lenb", [P, 1], F32, kind="ExternalInput")
    out = nc.dram_tensor("out", [T, D], F32, kind="ExternalOutput")
    nlen = nc.dram_tensor("nlen", [1, 1], I32, kind="ExternalOutput")
    with tile.TileContext(nc) as tc:
        with ExitStack() as ctx:
            _build_body(ctx, tc, nc, lg.ap(), hs.ap(), lenb.ap(), out.ap(),
                        nlen.ap())
    nc.compile()
    return nc


_NC = None


def _get_nc():
    global _NC
    if _NC is None:
        _NC = build_nc()
    return _NC


def make_in_maps(hidden_states, ctc_logits, lengths):
    in_maps = []
    for b in range(NCORES):
        in_maps.append({
            "lg": np.ascontiguousarray(ctc_logits[b], dtype=np.float32),
            "hs": np.ascontiguousarray(hidden_states[b], dtype=np.float32),
            "lenb": np.full((P, 1), float(lengths[b]), dtype=np.float32),
        })
    return in_maps


def kernel(hidden_states, ctc_logits, lengths, **run_kwargs):
    hidden_states = np.asarray(hidden_states)
    ctc_logits = np.asarray(ctc_logits)
    lengths = np.asarray(lengths)
    nc = _get_nc()
    in_maps = make_in_maps(hidden_states, ctc_logits, lengths)
    res = run_bass_kernel_spmd(nc, in_maps, core_ids=list(range(NCORES)),
                               **run_kwargs)
    compressed = np.stack([res.results[b]["out"] for b in range(NCORES)])
    new_lengths = np.array(
        [res.results[b]["nlen"].reshape(()) for b in range(NCORES)],
        dtype=np.int32)
    return compressed, new_lengths
